# revision 1
# baseline (speedup 1.0000x reference)
"""Distributed exact-kNN kernel for Trainium2 (8 NeuronCores).

Problem: B=2048 queries (512-d), N=100000 fitted rows, k=5 nearest
neighbors by squared L2; output = mean of the 5 neighbor vectors.

Strategy (sharding_hint: shard X_fit along N):
  - Each of the 8 cores holds a 12500-row shard of X_fit (transposed,
    fp8e4 DoubleRow layout) and all 2048 queries.
  - Device, per core: score s = 2 q.x - ||x||^2 (the per-query ||q||^2
    is constant within a row, so it cannot change the ranking).  The
    matmul accumulates 2 fp8 DoubleRow K-chunks of 2q.x PLUS a K=1
    bf16 "ones row" whose rhs carries (512 - ||x||^2), so PSUM holds
    the full score directly.  Epilogue, spread over three engines:
      ScalarE:  s32 = int32(relu(psum * 2 + 800))   # quantize, >= 0
                (relu matters: negative int32 packed values would
                 bitcast to f32 NaNs and poison max8)
      VectorE:  packed = s32 * 8192 + col_idx
                (top candidates stay < 2^24 so this is exact in f32)
      VectorE:  max8 over the f32-bitcast packed array -> top-8
                candidates WITH their indices, one op per piece.
  - Host: merge the 8*40 candidates per query, keep the best 16 by
    packed value, recompute exact f32 distances for those, pick the
    exact top-k and average the f32 vectors.  The exact re-rank makes
    the fp8/quantization noise irrelevant as long as the true top-5
    survive in the candidate set, which holds with huge margin
    (a true top-5 member is always within the top-5 of its own piece
    up to ~2-unit score noise against ~6-unit rank gaps; we keep 8 per
    piece and 40 per core).
"""

import sys

if "/opt/trn_rl_repo" not in sys.path:
    sys.path.insert(0, "/opt/trn_rl_repo")

import numpy as np
import ml_dtypes

# ---- problem geometry (hardcoded per spec) ----
B = 2048  # queries
D = 512  # feature dim
N = 100000  # fitted rows
NCORES = 8
NSHARD = N // NCORES  # 12500
NPAD = 12800  # padded shard length
PIECE = 2560  # shard processed in NPAD//PIECE pieces (SBUF residency)
QB = 128  # queries per block
NQB = B // QB  # 16
DCH = D // 128  # 4 contraction chunks
XOFF = 512.0  # xs row = XOFF - ||x||^2  (centered for bf16 accuracy)
XS_PAD = -900.0  # pad columns sink to the bottom
# Quantize-then-pack: s11 = int(relu(psum*2 + 800)) (quantum = 0.5 score units,
# top candidates ~<2047 so s11*8192+n < 2^24 stays EXACT in f32 arithmetic);
# packed = s11*8192 + col_idx via an all-arith DVE TensorScalarPtr.
SCALE_Q = 2.0
BIAS_Q = 800.0
PACK_MULT = 8192.0
PACK_MOD = 8192

_compiled = None


def _pack_op(nc, mybir, out, in0, in1, eng):
    """packed = (in0 * PACK_MULT) + in1 — all-arith TSP, runs on ACT or DVE."""
    return eng.add_instruction(
        mybir.InstTensorScalarPtr(
            name=nc.get_next_instruction_name(),
            is_scalar_tensor_tensor=True,
            op0=mybir.AluOpType.mult,
            op1=mybir.AluOpType.add,
            ins=[
                eng.lower_ap(in0),
                mybir.ImmediateValue(dtype=mybir.dt.float32, value=PACK_MULT),
                eng.lower_ap(in1),
            ],
            outs=[eng.lower_ap(out)],
        )
    )


FP8 = True  # use fp8e4 DoubleRow matmuls for the 2q.x term
WORK_BUFS = 8
ABLATE = ""  # dev: "dve" skips pack+max8, "act" skips epilogue, "mm" matmuls only
REPEAT = 1  # dev: run the whole pipeline N times (for overhead-cancelling timing)
POOL_PACK = 0  # out of 16: how many pack ops run on GpSimd (as 2 tensor_tensor ops)
ACT_GROUP = 1  # chunks per activation instruction (1 -> [128,512], 2 -> [128,1024])
PACK_INPLACE = True  # pack writes over the s32 tile (halves work-pool SBUF)


def _build():
    import concourse.mybir as mybir
    import concourse.tile as tile
    from concourse import bacc

    nc = bacc.Bacc(None, target_bir_lowering=False)

    xdt = mybir.dt.float8e4 if FP8 else mybir.dt.bfloat16
    if FP8:
        qT = nc.dram_tensor("qT", [4, 128, B], xdt, kind="ExternalInput")
        xT = nc.dram_tensor("xT", [4, 128, NPAD], xdt, kind="ExternalInput")
    else:
        qT = nc.dram_tensor("qT", [D, B], xdt, kind="ExternalInput")
        xT = nc.dram_tensor("xT", [D, NPAD], xdt, kind="ExternalInput")
    xs = nc.dram_tensor("xs", [1, NPAD], mybir.dt.bfloat16, kind="ExternalInput")
    ones = nc.dram_tensor("ones", [1, B], mybir.dt.bfloat16, kind="ExternalInput")
    iota = nc.dram_tensor("iota", [QB, PIECE], mybir.dt.int32, kind="ExternalInput")
    npieces = NPAD // PIECE
    cand = nc.dram_tensor(
        "cand", [B, 8 * npieces], mybir.dt.uint32, kind="ExternalOutput"
    )
    # matmul column chunks within a piece (<=512 each), grouped per ACT instr
    chunks = []
    off = 0
    while off < PIECE:
        w = min(512, PIECE - off)
        chunks.append((off, w))
        off += w
    groups = [chunks[i : i + ACT_GROUP] for i in range(0, len(chunks), ACT_GROUP)]

    with tile.TileContext(nc) as tc:
        with (
            tc.tile_pool(name="persist", bufs=1) as pp,
            tc.tile_pool(name="xpool", bufs=2) as xp_pool,
            tc.tile_pool(name="work", bufs=WORK_BUFS) as wp,
            tc.tile_pool(name="out", bufs=4) as op,
            tc.tile_pool(name="ps", bufs=8 // ACT_GROUP, space="PSUM") as ps,
        ):
            if FP8:
                qT_t = pp.tile([128, DCH, B], xdt, name="qTt")
                nc.sync.dma_start(
                    qT_t[:], qT[:].rearrange("c p b -> p c b")
                )
            else:
                qT_t = [
                    pp.tile([128, B], xdt, name=f"qTt{d}") for d in range(DCH)
                ]
                for d in range(DCH):
                    nc.sync.dma_start(qT_t[d][:], qT[d * 128 : (d + 1) * 128, :])
            xs_t = pp.tile([1, NPAD], mybir.dt.bfloat16, name="xs_t")
            ones_t = pp.tile([1, B], mybir.dt.bfloat16, name="ones_t")
            iota_t = pp.tile([QB, PIECE], mybir.dt.int32, name="iota_t")
            nc.sync.dma_start(xs_t[:], xs[:])
            nc.sync.dma_start(ones_t[:], ones[:])
            nc.sync.dma_start(iota_t[:], iota[:])
            c8192_t = None
            if POOL_PACK > 0:
                c8192_t = pp.tile([QB, PIECE], mybir.dt.int32, name="c8192_t")
                nc.gpsimd.memset(c8192_t[:], 8192)
            bias_t = pp.tile([QB, 1], mybir.dt.float32, name="bias_t")
            nc.vector.memset(bias_t[:], BIAS_Q)

            for rep in range(REPEAT):
              for p in range(npieces):
                lo = p * PIECE
                if FP8:
                    xp = xp_pool.tile([128, DCH, PIECE], xdt, tag="xp", name="xp")
                    nc.sync.dma_start(
                        xp[:],
                        xT[:, :, lo : lo + PIECE].rearrange("c p n -> p c n"),
                    )
                else:
                    xp = [
                        xp_pool.tile([128, PIECE], xdt, tag=f"xp{d}", name=f"xp{d}")
                        for d in range(DCH)
                    ]
                    for d in range(DCH):
                        nc.sync.dma_start(
                            xp[d][:], xT[d * 128 : (d + 1) * 128, lo : lo + PIECE]
                        )

                for qb in range(NQB):
                    s32_t = wp.tile([QB, PIECE], mybir.dt.int32, tag="s32", name="s32_t")
                    packed_t = (
                        s32_t
                        if PACK_INPLACE
                        else wp.tile(
                            [QB, PIECE], mybir.dt.int32, tag="packed", name="packed_t"
                        )
                    )
                    for grp in groups:
                        g_off, g_w = grp[0][0], sum(w for _, w in grp)
                        psum = ps.tile(
                            [QB, 512 * ACT_GROUP], mybir.dt.float32, tag="psum", name="psum"
                        )
                        for c_off, c_w in grp:
                            j = c_off - g_off
                            pslice = psum[:, j : j + c_w]
                            nc.tensor.matmul(
                                pslice,
                                ones_t[:, qb * QB : (qb + 1) * QB],
                                xs_t[:, lo + c_off : lo + c_off + c_w],
                                start=True,
                                stop=False,
                            )
                            if FP8:
                                for kk in range(0, DCH, 2):
                                    nc.tensor.matmul(
                                        pslice,
                                        qT_t[:, kk : kk + 2, qb * QB : (qb + 1) * QB],
                                        xp[:, kk : kk + 2, c_off : c_off + c_w],
                                        start=False,
                                        stop=(kk + 2 >= DCH),
                                        perf_mode=mybir.MatmulPerfMode.DoubleRow,
                                    )
                            else:
                                for d in range(DCH):
                                    nc.tensor.matmul(
                                        pslice,
                                        qT_t[d][:, qb * QB : (qb + 1) * QB],
                                        xp[d][:, c_off : c_off + c_w],
                                        start=False,
                                        stop=(d == DCH - 1),
                                    )
                        if ABLATE not in ("act", "mm"):
                            # Relu clamps negatives to 0: negative int32 packed
                            # values would bitcast to f32 NaNs and poison max8.
                            nc.scalar.activation(
                                out=s32_t[:, g_off : g_off + g_w],
                                in_=psum[:, :g_w],
                                func=mybir.ActivationFunctionType.Relu,
                                scale=SCALE_Q,
                                bias=bias_t[:],
                            )
                    if ABLATE in ("dve", "mm"):
                        continue
                    inst_i = (rep * npieces + p) * NQB + qb
                    if (inst_i % 16) < POOL_PACK:
                        # GpSimd path: packed = s32 * 8192 + iota  (2 ops)
                        nc.gpsimd.tensor_tensor(
                            packed_t[:], s32_t[:], c8192_t[:], mybir.AluOpType.mult
                        )
                        nc.gpsimd.tensor_tensor(
                            packed_t[:], packed_t[:], iota_t[:], mybir.AluOpType.add
                        )
                    else:
                        _pack_op(
                            nc,
                            mybir,
                            out=packed_t[:],
                            in0=s32_t[:],
                            in1=iota_t[:],
                            eng=nc.vector,
                        )
                    out8 = op.tile([QB, 8], mybir.dt.uint32, tag="out8", name="out8")
                    nc.vector.max(
                        out=out8[:].bitcast(mybir.dt.float32),
                        in_=packed_t[:].bitcast(mybir.dt.float32),
                    )
                    nc.sync.dma_start(
                        cand[qb * QB : (qb + 1) * QB, p * 8 : (p + 1) * 8], out8[:]
                    )
    nc.compile()
    return nc


def _get_compiled():
    global _compiled
    if _compiled is None:
        _compiled = _build()
    return _compiled


def _prepare_inputs(q, X):
    """Build per-core in_maps. q: [B, D] f32, X: [N, D] f32."""
    bf16 = ml_dtypes.bfloat16
    if FP8:
        from concourse import mybir

        xdt = mybir.dt.np(mybir.dt.float8e4)
        # [4, 128, B]: element [c, p, b] = 2*q[b, 128*c + p]
        qT_bf = np.ascontiguousarray(
            (2.0 * q).T.astype(xdt).reshape(DCH, 128, B)
        )
    else:
        xdt = bf16
        qT_bf = np.ascontiguousarray((2.0 * q).T.astype(bf16))
    ones_np = np.ones((1, B), dtype=bf16)
    iota_np = np.ascontiguousarray(
        np.broadcast_to(np.arange(PIECE, dtype=np.int32), (QB, PIECE))
    )
    in_maps = []
    for core in range(NCORES):
        Xi = X[core * NSHARD : (core + 1) * NSHARD]
        if FP8:
            xT_np = np.zeros((DCH, 128, NPAD), dtype=xdt)
            xT_np[:, :, :NSHARD] = Xi.T.astype(xdt).reshape(DCH, 128, NSHARD)
        else:
            xT_np = np.zeros((D, NPAD), dtype=xdt)
            xT_np[:, :NSHARD] = Xi.T.astype(xdt)
        xsq = np.einsum("nd,nd->n", Xi, Xi, dtype=np.float32)
        xs_row = np.full((1, NPAD), XS_PAD, dtype=np.float32)
        xs_row[0, :NSHARD] = XOFF - xsq
        in_maps.append(
            {
                "qT": qT_bf,
                "xT": xT_np,
                "xs": xs_row.astype(bf16),
                "ones": ones_np,
                "iota": iota_np,
            }
        )
    return in_maps


def _run_device(in_maps, trace=False, tmpdir=None):
    from concourse.bass_utils import run_bass_kernel_spmd

    nc = _get_compiled()
    kwargs = {}
    if trace:
        kwargs = {"trace": True, "tmpdir": tmpdir}
    return run_bass_kernel_spmd(
        nc, in_maps, core_ids=list(range(NCORES)), **kwargs
    )


def _merge_host(cand_all, q, X, k):
    """cand_all: [NCORES, B, ncand] uint32 (int32 packed). Returns [B, 1, D] f32."""
    ncand = cand_all.shape[2]
    packed = cand_all.astype(np.uint32).view(np.int32).astype(np.int64)
    pieces = (np.arange(ncand) // 8)[None, None, :]  # [1,1,ncand]
    local = packed % PACK_MOD  # column index (valid only for packed >= 0)
    within = pieces * PIECE + local  # [NCORES, B, CAND] position within shard
    gidx = np.arange(NCORES)[:, None, None] * NSHARD + within
    valid = (packed > 0) & (within < NSHARD)  # negatives/padding out
    packed = np.where(valid, packed, -1)
    gidx = np.where(valid, gidx, 0)

    packed_b = np.moveaxis(packed, 0, 1).reshape(B, NCORES * ncand)
    gidx_b = np.moveaxis(gidx, 0, 1).reshape(B, NCORES * ncand)

    C = max(16, 3 * k)
    top = np.argpartition(-packed_b, C, axis=1)[:, :C]  # [B, C]
    cidx = np.take_along_axis(gidx_b, top, axis=1)  # [B, C] global rows

    Xg = X[cidx]  # [B, C, D]
    xsq_g = np.einsum("bcd,bcd->bc", Xg, Xg, dtype=np.float32)
    d2 = xsq_g - 2.0 * np.einsum("bcd,bd->bc", Xg, q, dtype=np.float32)
    # guard (paranoia): invalid candidates, if any survived, go to +inf
    d2 = np.where(np.take_along_axis(packed_b, top, axis=1) < 0, np.inf, d2)
    win = np.argpartition(d2, k - 1, axis=1)[:, :k]  # [B, k]
    neigh = np.take_along_axis(Xg, win[:, :, None], axis=1)  # [B, k, D]
    return neigh.mean(axis=1, dtype=np.float32).reshape(B, 1, D).astype(np.float32)


def kernel(x_enc, X_fit, n_neighbors, _trace=False, _tmpdir=None):
    q = np.asarray(x_enc, dtype=np.float32).reshape(B, D)
    X = np.asarray(X_fit, dtype=np.float32)
    k = int(n_neighbors)
    in_maps = _prepare_inputs(q, X)
    res = _run_device(in_maps, trace=_trace, tmpdir=_tmpdir)
    cand_all = np.stack([res.results[c]["cand"] for c in range(NCORES)])
    out = _merge_host(cand_all, q, X, k)
    if _trace:
        return out, res
    return out



# revision 8
# speedup vs baseline: 3.9729x; 3.9729x over previous
"""Distributed exact-kNN kernel for Trainium2 (8 NeuronCores).

Problem: B=2048 queries (512-d), N=100000 fitted rows, k=5 nearest
neighbors by squared L2; output = mean of the 5 neighbor vectors.

Strategy (v2 — thresholded score map, no on-device top-k):
  - Shard X_fit by rows across 8 cores (12500 rows each, padded 12800).
  - Device computes s = 2 q.x - ||x||^2 - TAU for all (query, row) pairs
    via fp8e4 DoubleRow matmuls (the per-query ||q||^2 shifts every score
    of a row equally, so it cannot change the ranking), then emits an
    fp8e5 "survivor map": v > 0 iff s > TAU.  TAU is a global constant
    calibrated so each query keeps >= ~25 survivors out of 100k with a
    ~19-unit margin over the per-query 5th-best score (device score
    noise is ~2 units), so the true top-k always survive.
  - Two epilogue paths split the PSUM-eviction work across engines:
      A-tiles: a K=1 ones x (-||x||^2 - TAU) bf16 matmul folds the bias
               into PSUM; ScalarE Relu evicts to fp8e5 (0 = dead).
      B-tiles: PSUM holds only 2q.x; DVE tensor_tensor adds a broadcast
               bf16 bias row and converts to fp8e5 (sign = survivor bit).
      C-tiles: ScalarE copies PSUM->SBUF f32, GpSimd tensor_tensor adds
               the bias row -> fp8e5 (third engine in the rotation).
  - Host: scan the 8x[2048,12800] byte map for survivors (value in
    (0x00, 0x80)), re-rank candidates with exact f32 distances, take the
    exact top-k and average.  Queries with fewer than k survivors (never
    happens at the calibrated TAU) fall back to exact brute force.
"""

import sys

if "/opt/trn_rl_repo" not in sys.path:
    sys.path.insert(0, "/opt/trn_rl_repo")

import numpy as np
import ml_dtypes

# ---- problem geometry (hardcoded per spec) ----
B = 2048  # queries
D = 512  # feature dim
N = 100000  # fitted rows
NCORES = 8
NSHARD = N // NCORES  # 12500
NPAD = 12800  # padded shard length
PIECE = 2560  # shard processed in NPAD//PIECE pieces (SBUF residency)
QB = 128  # queries per block
NQB = B // QB  # 16
DCH = D // 128  # 4 contraction chunks
TAU = -347.0  # global survivor threshold on s = 2q.x - ||x||^2
XS_PAD = -30000.0  # pad columns sink far below any survivor

REPEAT = 1  # dev: run the whole pipeline N times (overhead-cancelling timing)
ABLATE = ""  # dev: "mm" = matmuls only (no epilogue, no map DMA)
# Per-512-col-group epilogue path schedule (cycled): "A" = ones-mm + ACT relu,
# "B" = DVE tensor_tensor add, "C" = ACT copy + GpSimd add.  Interleaving at
# group granularity keeps all three engines concurrently busy.
SCHEDULE = "ABCBABCBABCABBABCBAB"  # A=6 B=10 C=4 per 20 group-slots
WORK_BUFS = 8  # fp8 map tiles in flight
F32_BUFS = 4  # C-path f32 intermediate tiles in flight
PSUM_GROUP = 2  # 512-col chunks per PSUM tile (1 -> [128,512] x8, 2 -> [128,1024] x4)

_compiled = None


def _build():
    import concourse.mybir as mybir
    import concourse.tile as tile
    from concourse import bacc

    nc = bacc.Bacc(None, target_bir_lowering=False)

    xdt = mybir.dt.float8e4
    qT = nc.dram_tensor("qT", [4, 128, B], xdt, kind="ExternalInput")
    xT = nc.dram_tensor("xT", [4, 128, NPAD], xdt, kind="ExternalInput")
    # A-path: K=1 matmul row carrying -||x||^2 - TAU
    xsA = nc.dram_tensor("xsA", [1, NPAD], mybir.dt.bfloat16, kind="ExternalInput")
    ones = nc.dram_tensor("ones", [1, B], mybir.dt.bfloat16, kind="ExternalInput")
    # B/C-path: the same row materialized across 128 partitions
    xsB = nc.dram_tensor("xsB", [QB, NPAD], mybir.dt.bfloat16, kind="ExternalInput")
    npieces = NPAD // PIECE
    smap = nc.dram_tensor("smap", [B, NPAD], mybir.dt.float8e5, kind="ExternalOutput")

    gw = 512 * PSUM_GROUP  # columns per PSUM tile
    groups = [(o, min(gw, PIECE - o)) for o in range(0, PIECE, gw)]

    with tile.TileContext(nc) as tc:
        with (
            tc.tile_pool(name="persist", bufs=1) as pp,
            tc.tile_pool(name="xpool", bufs=2) as xp_pool,
            tc.tile_pool(name="work", bufs=WORK_BUFS) as wp,
            tc.tile_pool(name="f32w", bufs=F32_BUFS) as fp,
            tc.tile_pool(name="ps", bufs=8 // PSUM_GROUP, space="PSUM") as ps,
        ):
            qT_t = pp.tile([128, DCH, B], xdt, name="qTt")
            nc.sync.dma_start(qT_t[:], qT[:].rearrange("c p b -> p c b"))
            xsA_t = pp.tile([1, NPAD], mybir.dt.bfloat16, name="xsA_t")
            ones_t = pp.tile([1, B], mybir.dt.bfloat16, name="ones_t")
            xsB_t = pp.tile([QB, NPAD], mybir.dt.bfloat16, name="xsB_t")
            nc.sync.dma_start(xsA_t[:], xsA[:])
            nc.sync.dma_start(ones_t[:], ones[:])
            nc.sync.dma_start(xsB_t[:], xsB[:])

            for rep in range(REPEAT):
              for p in range(npieces):
                lo = p * PIECE
                xp = xp_pool.tile([128, DCH, PIECE], xdt, tag="xp", name="xp")
                nc.sync.dma_start(
                    xp[:], xT[:, :, lo : lo + PIECE].rearrange("c p n -> p c n")
                )

                for qb in range(NQB):
                    m_t = wp.tile(
                        [QB, PIECE], mybir.dt.float8e5, tag="map", name="m_t"
                    )
                    f_t = None
                    for gi, (c_off, c_w) in enumerate(groups):
                        gslot = (p * NQB + qb) * len(groups) + gi
                        path = SCHEDULE[gslot % len(SCHEDULE)]
                        psum = ps.tile([QB, gw], mybir.dt.float32, tag="ps", name="ps")
                        for s_off in range(0, c_w, 512):
                            s_w = min(512, c_w - s_off)
                            pslice = psum[:, s_off : s_off + s_w]
                            a_off = lo + c_off + s_off
                            if path == "A":
                                nc.tensor.matmul(
                                    pslice,
                                    ones_t[:, qb * QB : (qb + 1) * QB],
                                    xsA_t[:, a_off : a_off + s_w],
                                    start=True,
                                    stop=False,
                                )
                            for kk in range(0, DCH, 2):
                                nc.tensor.matmul(
                                    pslice,
                                    qT_t[:, kk : kk + 2, qb * QB : (qb + 1) * QB],
                                    xp[:, kk : kk + 2, c_off + s_off : c_off + s_off + s_w],
                                    start=(path != "A" and kk == 0),
                                    stop=(kk + 2 >= DCH),
                                    perf_mode=mybir.MatmulPerfMode.DoubleRow,
                                )
                        if ABLATE == "mm":
                            continue
                        dst = m_t[:, c_off : c_off + c_w]
                        if path == "A":
                            nc.scalar.activation(
                                out=dst,
                                in_=psum[:, :c_w],
                                func=mybir.ActivationFunctionType.Relu,
                            )
                        elif path == "B":
                            nc.vector.tensor_tensor(
                                dst,
                                psum[:, :c_w],
                                xsB_t[:, lo + c_off : lo + c_off + c_w],
                                mybir.AluOpType.add,
                            )
                        else:  # C
                            if f_t is None:
                                f_t = fp.tile(
                                    [QB, PIECE], mybir.dt.float32, tag="f32", name="f_t"
                                )
                            nc.scalar.activation(
                                out=f_t[:, c_off : c_off + c_w],
                                in_=psum[:, :c_w],
                                func=mybir.ActivationFunctionType.Copy,
                            )
                            nc.gpsimd.tensor_tensor(
                                dst,
                                f_t[:, c_off : c_off + c_w],
                                xsB_t[:, lo + c_off : lo + c_off + c_w],
                                mybir.AluOpType.add,
                            )
                    if ABLATE == "mm":
                        continue
                    nc.sync.dma_start(
                        smap[qb * QB : (qb + 1) * QB, lo : lo + PIECE], m_t[:]
                    )
    nc.compile()
    return nc


def _get_compiled():
    global _compiled
    if _compiled is None:
        _compiled = _build()
    return _compiled


def _prepare_inputs(q, X):
    """Build per-core in_maps. q: [B, D] f32, X: [N, D] f32."""
    bf16 = ml_dtypes.bfloat16
    from concourse import mybir

    xdt = mybir.dt.np(mybir.dt.float8e4)
    # [4, 128, B]: element [c, p, b] = 2*q[b, 128*c + p]
    qT_f8 = np.ascontiguousarray((2.0 * q).T.astype(xdt).reshape(DCH, 128, B))
    ones_np = np.ones((1, B), dtype=bf16)
    in_maps = []
    for core in range(NCORES):
        Xi = X[core * NSHARD : (core + 1) * NSHARD]
        xT_np = np.zeros((DCH, 128, NPAD), dtype=xdt)
        xT_np[:, :, :NSHARD] = Xi.T.astype(xdt).reshape(DCH, 128, NSHARD)
        xsq = np.einsum("nd,nd->n", Xi, Xi, dtype=np.float32)
        xs_row = np.full((1, NPAD), XS_PAD, dtype=np.float32)
        xs_row[0, :NSHARD] = -xsq - TAU
        xs_bf = xs_row.astype(bf16)
        in_maps.append(
            {
                "qT": qT_f8,
                "xT": xT_np,
                "xsA": xs_bf,
                "ones": ones_np,
                "xsB": np.ascontiguousarray(np.broadcast_to(xs_bf, (QB, NPAD))),
            }
        )
    return in_maps


def _run_device(in_maps, trace=False, tmpdir=None):
    from concourse.bass_utils import run_bass_kernel_spmd

    nc = _get_compiled()
    kwargs = {}
    if trace:
        kwargs = {"trace": True, "tmpdir": tmpdir}
    return run_bass_kernel_spmd(nc, in_maps, core_ids=list(range(NCORES)), **kwargs)


def _merge_host(maps_u8, q, X, k):
    """maps_u8: [NCORES, B, NPAD] uint8 view of the fp8e5 survivor map.
    Returns [B, 1, D] f32: mean of the exact top-k neighbor rows."""
    xsq_all = np.einsum("nd,nd->n", X, X, dtype=np.float32)
    # survivor = positive fp8e5 byte (0x00 < b < 0x80), real column only
    alive = (maps_u8 > 0) & (maps_u8 < 0x80)
    alive[:, :, NSHARD:] = False
    core_i, q_i, n_i = np.nonzero(alive)
    g_i = core_i * NSHARD + n_i  # global X row
    order = np.argsort(q_i, kind="stable")
    q_s, g_s = q_i[order], g_i[order]
    counts = np.bincount(q_s, minlength=B)
    offs = np.concatenate([[0], np.cumsum(counts)])

    out = np.empty((B, D), dtype=np.float32)
    M = int(counts.max()) if counts.size else 0
    lowq = np.nonzero(counts < k)[0]
    for qi in lowq:  # safety net: brute-force deficient queries (expected: none)
        d2 = xsq_all - 2.0 * (X @ q[qi])
        idx = np.argpartition(d2, k - 1)[:k]
        out[qi] = X[idx].mean(axis=0, dtype=np.float32)

    # vectorized exact re-rank in query chunks
    CH = 256
    pad_idx = np.zeros((B, M), dtype=np.int64)
    valid = np.zeros((B, M), dtype=bool)
    for qi in range(B):
        c = counts[qi]
        if c:
            pad_idx[qi, :c] = g_s[offs[qi] : offs[qi] + c]
            valid[qi, :c] = True
    for i in range(0, B, CH):
        sl = slice(i, i + CH)
        cid = pad_idx[sl]  # [CH, M]
        Xg = X[cid]  # [CH, M, D]
        d2 = xsq_all[cid] - 2.0 * np.einsum(
            "cmd,cd->cm", Xg, q[sl], dtype=np.float32, optimize=True
        )
        d2[~valid[sl]] = np.inf
        for j in range(cid.shape[0]):
            qi = i + j
            if counts[qi] < k:
                continue  # already brute-forced
            win = np.argpartition(d2[j], k - 1)[:k]
            out[qi] = Xg[j, win].mean(axis=0, dtype=np.float32)
    return out.reshape(B, 1, D)


def kernel(x_enc, X_fit, n_neighbors, _trace=False, _tmpdir=None):
    q = np.asarray(x_enc, dtype=np.float32).reshape(B, D)
    X = np.asarray(X_fit, dtype=np.float32)
    k = int(n_neighbors)
    in_maps = _prepare_inputs(q, X)
    res = _run_device(in_maps, trace=_trace, tmpdir=_tmpdir)
    maps = np.stack(
        [res.results[c]["smap"].view(np.uint8) for c in range(NCORES)]
    ).reshape(NCORES, B, NPAD)
    out = _merge_host(maps, q, X, k)
    if _trace:
        return out, res
    return out


# revision 12
# speedup vs baseline: 6.5292x; 1.6434x over previous
"""Distributed exact-kNN kernel for Trainium2 (8 NeuronCores).

Problem: B=2048 queries (512-d), N=100000 fitted rows, k=5 nearest
neighbors by squared L2; output = mean of the 5 neighbor vectors.

Strategy (v2 — thresholded score map, no on-device top-k):
  - Shard X_fit by rows across 8 cores (12500 rows each, padded 12800).
  - Device computes s = 2 q.x - ||x||^2 - TAU for all (query, row) pairs
    via fp8e4 DoubleRow matmuls (the per-query ||q||^2 shifts every score
    of a row equally, so it cannot change the ranking), then emits an
    fp8e5 "survivor map": v > 0 iff s > TAU.  TAU is a global constant
    calibrated so each query keeps >= ~25 survivors out of 100k with a
    ~19-unit margin over the per-query 5th-best score (device score
    noise is ~2 units), so the true top-k always survive.
  - Two epilogue paths split the PSUM-eviction work across engines:
      A-tiles: a K=1 ones x (-||x||^2 - TAU) bf16 matmul folds the bias
               into PSUM; ScalarE Relu evicts to fp8e5 (0 = dead).
      B-tiles: PSUM holds only 2q.x; DVE tensor_tensor adds a broadcast
               bf16 bias row and converts to fp8e5 (sign = survivor bit).
      C-tiles: ScalarE copies PSUM->SBUF f32, GpSimd tensor_tensor adds
               the bias row -> fp8e5 (third engine in the rotation).
  - Host: scan the 8x[2048,12800] byte map for survivors (value in
    (0x00, 0x80)), re-rank candidates with exact f32 distances, take the
    exact top-k and average.  Queries with fewer than k survivors (never
    happens at the calibrated TAU) fall back to exact brute force.
"""

import sys

if "/opt/trn_rl_repo" not in sys.path:
    sys.path.insert(0, "/opt/trn_rl_repo")

import numpy as np
import ml_dtypes

# ---- problem geometry (hardcoded per spec) ----
B = 2048  # queries
D = 512  # feature dim
N = 100000  # fitted rows
NCORES = 8
NSHARD = N // NCORES  # 12500
NPAD = 12800  # padded shard length
PIECE = 2560  # shard processed in NPAD//PIECE pieces (SBUF residency)
QB = 128  # queries per block
NQB = B // QB  # 16
DCH = D // 128  # 4 contraction chunks
TAU = -347.0  # global survivor threshold on s = 2q.x - ||x||^2
XS_PAD = -30000.0  # pad columns sink far below any survivor

REPEAT = 1  # dev: run the whole pipeline N times (overhead-cancelling timing)
ABLATE = ""  # dev: "mm" = matmuls only (no epilogue, no map DMA)
# Per-512-col-group epilogue path schedule (cycled): "A" = ones-mm + ACT relu,
# "B" = DVE tensor_tensor add, "C" = ACT copy + GpSimd add.  Interleaving at
# group granularity keeps all three engines concurrently busy.
SCHEDULE = "ABCBABCBABCABBABCBAB"  # A=6 B=10 C=4 per 20 group-slots
WORK_BUFS = 8  # fp8 map tiles in flight
F32_BUFS = 4  # C-path f32 intermediate tiles in flight
PSUM_GROUP = 2  # 512-col chunks per PSUM tile (1 -> [128,512] x8, 2 -> [128,1024] x4)

_compiled = None


def _build():
    import concourse.mybir as mybir
    import concourse.tile as tile
    from concourse import bacc

    nc = bacc.Bacc(None, target_bir_lowering=False)

    xdt = mybir.dt.float8e4
    qT = nc.dram_tensor("qT", [4, 128, B], xdt, kind="ExternalInput")
    xT = nc.dram_tensor("xT", [4, 128, NPAD], xdt, kind="ExternalInput")
    # A-path: K=1 matmul row carrying -||x||^2 - TAU
    xsA = nc.dram_tensor("xsA", [1, NPAD], mybir.dt.bfloat16, kind="ExternalInput")
    ones = nc.dram_tensor("ones", [1, B], mybir.dt.bfloat16, kind="ExternalInput")
    # B/C-path: the same row materialized across 128 partitions
    xsB = nc.dram_tensor("xsB", [QB, NPAD], mybir.dt.bfloat16, kind="ExternalInput")
    npieces = NPAD // PIECE
    smap = nc.dram_tensor("smap", [B, NPAD], mybir.dt.float8e5, kind="ExternalOutput")

    gw = 512 * PSUM_GROUP  # columns per PSUM tile
    groups = [(o, min(gw, PIECE - o)) for o in range(0, PIECE, gw)]

    with tile.TileContext(nc) as tc:
        with (
            tc.tile_pool(name="persist", bufs=1) as pp,
            tc.tile_pool(name="xpool", bufs=2) as xp_pool,
            tc.tile_pool(name="work", bufs=WORK_BUFS) as wp,
            tc.tile_pool(name="f32w", bufs=F32_BUFS) as fp,
            tc.tile_pool(name="ps", bufs=8 // PSUM_GROUP, space="PSUM") as ps,
        ):
            qT_t = pp.tile([128, DCH, B], xdt, name="qTt")
            nc.gpsimd.dma_start(qT_t[:], qT[:].rearrange("c p b -> p c b"))
            xsA_t = pp.tile([1, NPAD], mybir.dt.bfloat16, name="xsA_t")
            ones_t = pp.tile([1, B], mybir.dt.bfloat16, name="ones_t")
            nc.gpsimd.dma_start(xsA_t[:], xsA[:])
            nc.gpsimd.dma_start(ones_t[:], ones[:])

            out_qs = [nc.sync]
            for rep in range(REPEAT):
              for p in range(npieces):
                lo = p * PIECE
                xp = xp_pool.tile([128, DCH, PIECE], xdt, tag="xp", name="xp")
                nc.sync.dma_start(
                    xp[:], xT[:, :, lo : lo + PIECE].rearrange("c p n -> p c n")
                )
                xsB_t = xp_pool.tile(
                    [QB, PIECE], mybir.dt.bfloat16, tag="xsb", name="xsb"
                )
                nc.gpsimd.dma_start(xsB_t[:], xsB[:, lo : lo + PIECE])

                for qb in range(NQB):
                    m_t = wp.tile(
                        [QB, PIECE], mybir.dt.float8e5, tag="map", name="m_t"
                    )
                    f_t = None
                    for gi, (c_off, c_w) in enumerate(groups):
                        gslot = (p * NQB + qb) * len(groups) + gi
                        path = SCHEDULE[gslot % len(SCHEDULE)]
                        psum = ps.tile([QB, gw], mybir.dt.float32, tag="ps", name="ps")
                        for s_off in range(0, c_w, 512):
                            s_w = min(512, c_w - s_off)
                            pslice = psum[:, s_off : s_off + s_w]
                            a_off = lo + c_off + s_off
                            if path == "A":
                                nc.tensor.matmul(
                                    pslice,
                                    ones_t[:, qb * QB : (qb + 1) * QB],
                                    xsA_t[:, a_off : a_off + s_w],
                                    start=True,
                                    stop=False,
                                )
                            for kk in range(0, DCH, 2):
                                nc.tensor.matmul(
                                    pslice,
                                    qT_t[:, kk : kk + 2, qb * QB : (qb + 1) * QB],
                                    xp[:, kk : kk + 2, c_off + s_off : c_off + s_off + s_w],
                                    start=(path != "A" and kk == 0),
                                    stop=(kk + 2 >= DCH),
                                    perf_mode=mybir.MatmulPerfMode.DoubleRow,
                                )
                        if ABLATE == "mm":
                            continue
                        dst = m_t[:, c_off : c_off + c_w]
                        if path == "A":
                            nc.scalar.activation(
                                out=dst,
                                in_=psum[:, :c_w],
                                func=mybir.ActivationFunctionType.Relu,
                            )
                        elif path == "B":
                            nc.vector.tensor_tensor(
                                dst,
                                psum[:, :c_w],
                                xsB_t[:, c_off : c_off + c_w],
                                mybir.AluOpType.add,
                            )
                        else:  # C
                            if f_t is None:
                                f_t = fp.tile(
                                    [QB, PIECE], mybir.dt.float32, tag="f32", name="f_t"
                                )
                            nc.scalar.activation(
                                out=f_t[:, c_off : c_off + c_w],
                                in_=psum[:, :c_w],
                                func=mybir.ActivationFunctionType.Copy,
                            )
                            nc.gpsimd.tensor_tensor(
                                dst,
                                f_t[:, c_off : c_off + c_w],
                                xsB_t[:, c_off : c_off + c_w],
                                mybir.AluOpType.add,
                            )
                    if ABLATE == "mm":
                        continue
                    out_qs[qb % len(out_qs)].dma_start(
                        smap[qb * QB : (qb + 1) * QB, lo : lo + PIECE], m_t[:]
                    )
    nc.compile()
    return nc


def _get_compiled():
    global _compiled
    if _compiled is None:
        _compiled = _build()
    return _compiled


def _prepare_inputs(q, X):
    """Build per-core in_maps. q: [B, D] f32, X: [N, D] f32."""
    bf16 = ml_dtypes.bfloat16
    from concourse import mybir

    xdt = mybir.dt.np(mybir.dt.float8e4)
    # [4, 128, B]: element [c, p, b] = 2*q[b, 128*c + p]
    qT_f8 = np.ascontiguousarray((2.0 * q).T.astype(xdt).reshape(DCH, 128, B))
    ones_np = np.ones((1, B), dtype=bf16)
    in_maps = []
    for core in range(NCORES):
        Xi = X[core * NSHARD : (core + 1) * NSHARD]
        xT_np = np.zeros((DCH, 128, NPAD), dtype=xdt)
        xT_np[:, :, :NSHARD] = Xi.T.astype(xdt).reshape(DCH, 128, NSHARD)
        xsq = np.einsum("nd,nd->n", Xi, Xi, dtype=np.float32)
        xs_row = np.full((1, NPAD), XS_PAD, dtype=np.float32)
        xs_row[0, :NSHARD] = -xsq - TAU
        xs_bf = xs_row.astype(bf16)
        in_maps.append(
            {
                "qT": qT_f8,
                "xT": xT_np,
                "xsA": xs_bf,
                "ones": ones_np,
                "xsB": np.ascontiguousarray(np.broadcast_to(xs_bf, (QB, NPAD))),
            }
        )
    return in_maps


def _run_device(in_maps, trace=False, tmpdir=None):
    from concourse.bass_utils import run_bass_kernel_spmd

    nc = _get_compiled()
    kwargs = {}
    if trace:
        kwargs = {"trace": True, "tmpdir": tmpdir}
    return run_bass_kernel_spmd(nc, in_maps, core_ids=list(range(NCORES)), **kwargs)


def _merge_host(maps_u8, q, X, k):
    """maps_u8: [NCORES, B, NPAD] uint8 view of the fp8e5 survivor map.
    Returns [B, 1, D] f32: mean of the exact top-k neighbor rows."""
    xsq_all = np.einsum("nd,nd->n", X, X, dtype=np.float32)
    # survivor = positive fp8e5 byte (0x00 < b < 0x80), real column only
    alive = (maps_u8 > 0) & (maps_u8 < 0x80)
    alive[:, :, NSHARD:] = False
    core_i, q_i, n_i = np.nonzero(alive)
    g_i = core_i * NSHARD + n_i  # global X row
    order = np.argsort(q_i, kind="stable")
    q_s, g_s = q_i[order], g_i[order]
    counts = np.bincount(q_s, minlength=B)
    offs = np.concatenate([[0], np.cumsum(counts)])

    out = np.empty((B, D), dtype=np.float32)
    M = int(counts.max()) if counts.size else 0
    lowq = np.nonzero(counts < k)[0]
    for qi in lowq:  # safety net: brute-force deficient queries (expected: none)
        d2 = xsq_all - 2.0 * (X @ q[qi])
        idx = np.argpartition(d2, k - 1)[:k]
        out[qi] = X[idx].mean(axis=0, dtype=np.float32)

    # vectorized exact re-rank in query chunks
    CH = 256
    pad_idx = np.zeros((B, M), dtype=np.int64)
    valid = np.zeros((B, M), dtype=bool)
    for qi in range(B):
        c = counts[qi]
        if c:
            pad_idx[qi, :c] = g_s[offs[qi] : offs[qi] + c]
            valid[qi, :c] = True
    for i in range(0, B, CH):
        sl = slice(i, i + CH)
        cid = pad_idx[sl]  # [CH, M]
        Xg = X[cid]  # [CH, M, D]
        d2 = xsq_all[cid] - 2.0 * np.einsum(
            "cmd,cd->cm", Xg, q[sl], dtype=np.float32, optimize=True
        )
        d2[~valid[sl]] = np.inf
        for j in range(cid.shape[0]):
            qi = i + j
            if counts[qi] < k:
                continue  # already brute-forced
            win = np.argpartition(d2[j], k - 1)[:k]
            out[qi] = Xg[j, win].mean(axis=0, dtype=np.float32)
    return out.reshape(B, 1, D)


def kernel(x_enc, X_fit, n_neighbors, _trace=False, _tmpdir=None):
    q = np.asarray(x_enc, dtype=np.float32).reshape(B, D)
    X = np.asarray(X_fit, dtype=np.float32)
    k = int(n_neighbors)
    in_maps = _prepare_inputs(q, X)
    res = _run_device(in_maps, trace=_trace, tmpdir=_tmpdir)
    maps = np.stack(
        [res.results[c]["smap"].view(np.uint8) for c in range(NCORES)]
    ).reshape(NCORES, B, NPAD)
    out = _merge_host(maps, q, X, k)
    if _trace:
        return out, res
    return out


# revision 15
# speedup vs baseline: 17.4914x; 2.6789x over previous
"""Distributed exact-kNN kernel for Trainium2 (8 NeuronCores).

Problem: B=2048 queries (512-d), N=100000 fitted rows, k=5 nearest
neighbors by squared L2; output = mean of the 5 neighbor vectors.

Strategy (v2 — thresholded score map, no on-device top-k):
  - Shard X_fit by rows across 8 cores (12500 rows each, padded 12800).
  - Device computes s = 2 q.x - ||x||^2 - TAU for all (query, row) pairs
    via fp8e4 DoubleRow matmuls (the per-query ||q||^2 shifts every score
    of a row equally, so it cannot change the ranking), then emits an
    fp8e5 "survivor map": v > 0 iff s > TAU.  TAU is a global constant
    calibrated so each query keeps >= ~25 survivors out of 100k with a
    ~19-unit margin over the per-query 5th-best score (device score
    noise is ~2 units), so the true top-k always survive.
  - Two epilogue paths split the PSUM-eviction work across engines:
      A-tiles: a K=1 ones x (-||x||^2 - TAU) bf16 matmul folds the bias
               into PSUM; ScalarE Relu evicts to fp8e5 (0 = dead).
      B-tiles: PSUM holds only 2q.x; DVE tensor_tensor adds a broadcast
               bf16 bias row and converts to fp8e5 (sign = survivor bit).
      C-tiles: ScalarE copies PSUM->SBUF f32, GpSimd tensor_tensor adds
               the bias row -> fp8e5 (third engine in the rotation).
  - Host: scan the 8x[2048,12800] byte map for survivors (value in
    (0x00, 0x80)), re-rank candidates with exact f32 distances, take the
    exact top-k and average.  Queries with fewer than k survivors (never
    happens at the calibrated TAU) fall back to exact brute force.
"""

import sys

if "/opt/trn_rl_repo" not in sys.path:
    sys.path.insert(0, "/opt/trn_rl_repo")

import numpy as np
import ml_dtypes

# ---- problem geometry (hardcoded per spec) ----
B = 2048  # queries
D = 512  # feature dim
N = 100000  # fitted rows
NCORES = 8
NSHARD = N // NCORES  # 12500
NPAD = 12800  # padded shard length
PIECE = 2560  # shard processed in NPAD//PIECE pieces (SBUF residency)
QB = 128  # queries per block
NQB = B // QB  # 16
DCH = D // 128  # 4 contraction chunks
TAU = -347.0  # global survivor threshold on s = 2q.x - ||x||^2
XS_PAD = -30000.0  # pad columns sink far below any survivor

REPEAT = 1  # dev: run the whole pipeline N times (overhead-cancelling timing)
ABLATE = ""  # dev: "mm" = matmuls only (no epilogue, no map DMA)
SMALL_OUT = False  # dev/bench: smap becomes Internal (tiny "ok" output instead)
                   # so per-dispatch host I/O stops drowning the timing signal
# Per-512-col-group epilogue path schedule (cycled): "A" = ones-mm + ACT relu,
# "B" = DVE tensor_tensor add, "C" = ACT copy + GpSimd add.  Interleaving at
# group granularity keeps all three engines concurrently busy.
SCHEDULE = "ABCBABCBABCABBABCBAB"  # A=6 B=10 C=4 per 20 group-slots
WORK_BUFS = 8  # fp8 map tiles in flight
F32_BUFS = 4  # C-path f32 intermediate tiles in flight
PSUM_GROUP = 2  # 512-col chunks per PSUM tile (1 -> [128,512] x8, 2 -> [128,1024] x4)

_compiled = None


def _build():
    import concourse.mybir as mybir
    import concourse.tile as tile
    from concourse import bacc

    nc = bacc.Bacc(None, target_bir_lowering=False)

    xdt = mybir.dt.float8e4
    qT = nc.dram_tensor("qT", [4, 128, B], xdt, kind="ExternalInput")
    xT = nc.dram_tensor("xT", [4, 128, NPAD], xdt, kind="ExternalInput")
    # A-path: K=1 matmul row carrying -||x||^2 - TAU
    xsA = nc.dram_tensor("xsA", [1, NPAD], mybir.dt.bfloat16, kind="ExternalInput")
    ones = nc.dram_tensor("ones", [1, B], mybir.dt.bfloat16, kind="ExternalInput")
    # B/C-path: the same row materialized across 128 partitions
    xsB = nc.dram_tensor("xsB", [QB, NPAD], mybir.dt.bfloat16, kind="ExternalInput")
    npieces = NPAD // PIECE
    smap = nc.dram_tensor(
        "smap",
        [B, NPAD],
        mybir.dt.float8e5,
        kind="Internal" if SMALL_OUT else "ExternalOutput",
    )
    ok = (
        nc.dram_tensor("ok", [1, 4], mybir.dt.float32, kind="ExternalOutput")
        if SMALL_OUT
        else None
    )

    gw = 512 * PSUM_GROUP  # columns per PSUM tile
    groups = [(o, min(gw, PIECE - o)) for o in range(0, PIECE, gw)]

    with tile.TileContext(nc) as tc:
        with (
            tc.tile_pool(name="persist", bufs=1) as pp,
            tc.tile_pool(name="xpool", bufs=2) as xp_pool,
            tc.tile_pool(name="work", bufs=WORK_BUFS) as wp,
            tc.tile_pool(name="f32w", bufs=F32_BUFS) as fp,
            tc.tile_pool(name="ps", bufs=8 // PSUM_GROUP, space="PSUM") as ps,
        ):
            qT_t = pp.tile([128, DCH, B], xdt, name="qTt")
            nc.gpsimd.dma_start(qT_t[:], qT[:].rearrange("c p b -> p c b"))
            xsA_t = pp.tile([1, NPAD], mybir.dt.bfloat16, name="xsA_t")
            ones_t = pp.tile([1, B], mybir.dt.bfloat16, name="ones_t")
            nc.gpsimd.dma_start(xsA_t[:], xsA[:])
            nc.gpsimd.dma_start(ones_t[:], ones[:])

            out_qs = [nc.sync]
            for rep in range(REPEAT):
              for p in range(npieces):
                lo = p * PIECE
                xp = xp_pool.tile([128, DCH, PIECE], xdt, tag="xp", name="xp")
                nc.sync.dma_start(
                    xp[:], xT[:, :, lo : lo + PIECE].rearrange("c p n -> p c n")
                )
                xsB_t = xp_pool.tile(
                    [QB, PIECE], mybir.dt.bfloat16, tag="xsb", name="xsb"
                )
                nc.gpsimd.dma_start(xsB_t[:], xsB[:, lo : lo + PIECE])

                for qb in range(NQB):
                    m_t = wp.tile(
                        [QB, PIECE], mybir.dt.float8e5, tag="map", name="m_t"
                    )
                    f_t = None
                    for gi, (c_off, c_w) in enumerate(groups):
                        gslot = (p * NQB + qb) * len(groups) + gi
                        path = SCHEDULE[gslot % len(SCHEDULE)]
                        psum = ps.tile([QB, gw], mybir.dt.float32, tag="ps", name="ps")
                        for s_off in range(0, c_w, 512):
                            s_w = min(512, c_w - s_off)
                            pslice = psum[:, s_off : s_off + s_w]
                            a_off = lo + c_off + s_off
                            if path == "A":
                                nc.tensor.matmul(
                                    pslice,
                                    ones_t[:, qb * QB : (qb + 1) * QB],
                                    xsA_t[:, a_off : a_off + s_w],
                                    start=True,
                                    stop=False,
                                )
                            for kk in range(0, DCH, 2):
                                nc.tensor.matmul(
                                    pslice,
                                    qT_t[:, kk : kk + 2, qb * QB : (qb + 1) * QB],
                                    xp[:, kk : kk + 2, c_off + s_off : c_off + s_off + s_w],
                                    start=(path != "A" and kk == 0),
                                    stop=(kk + 2 >= DCH),
                                    perf_mode=mybir.MatmulPerfMode.DoubleRow,
                                )
                        if ABLATE == "mm":
                            continue
                        dst = m_t[:, c_off : c_off + c_w]
                        if path == "A":
                            nc.scalar.activation(
                                out=dst,
                                in_=psum[:, :c_w],
                                func=mybir.ActivationFunctionType.Relu,
                            )
                        elif path == "B":
                            nc.vector.tensor_tensor(
                                dst,
                                psum[:, :c_w],
                                xsB_t[:, c_off : c_off + c_w],
                                mybir.AluOpType.add,
                            )
                        else:  # C
                            if f_t is None:
                                f_t = fp.tile(
                                    [QB, PIECE], mybir.dt.float32, tag="f32", name="f_t"
                                )
                            nc.scalar.activation(
                                out=f_t[:, c_off : c_off + c_w],
                                in_=psum[:, :c_w],
                                func=mybir.ActivationFunctionType.Copy,
                            )
                            nc.gpsimd.tensor_tensor(
                                dst,
                                f_t[:, c_off : c_off + c_w],
                                xsB_t[:, c_off : c_off + c_w],
                                mybir.AluOpType.add,
                            )
                    if ABLATE == "mm":
                        continue
                    out_qs[qb % len(out_qs)].dma_start(
                        smap[qb * QB : (qb + 1) * QB, lo : lo + PIECE], m_t[:]
                    )
            if ok is not None:
                ok_t = pp.tile([1, 4], mybir.dt.float32, name="ok_t")
                nc.vector.memset(ok_t[:], 1.0)
                nc.sync.dma_start(ok[:], ok_t[:])
    nc.compile()
    return nc


def _get_compiled():
    global _compiled
    if _compiled is None:
        _compiled = _build()
    return _compiled


def _prepare_inputs(q, X):
    """Build per-core in_maps. q: [B, D] f32, X: [N, D] f32."""
    bf16 = ml_dtypes.bfloat16
    from concourse import mybir

    xdt = mybir.dt.np(mybir.dt.float8e4)
    # [4, 128, B]: element [c, p, b] = 2*q[b, 128*c + p]
    qT_f8 = np.ascontiguousarray((2.0 * q).T.astype(xdt).reshape(DCH, 128, B))
    ones_np = np.ones((1, B), dtype=bf16)
    in_maps = []
    for core in range(NCORES):
        Xi = X[core * NSHARD : (core + 1) * NSHARD]
        xT_np = np.zeros((DCH, 128, NPAD), dtype=xdt)
        xT_np[:, :, :NSHARD] = Xi.T.astype(xdt).reshape(DCH, 128, NSHARD)
        xsq = np.einsum("nd,nd->n", Xi, Xi, dtype=np.float32)
        xs_row = np.full((1, NPAD), XS_PAD, dtype=np.float32)
        xs_row[0, :NSHARD] = -xsq - TAU
        xs_bf = xs_row.astype(bf16)
        in_maps.append(
            {
                "qT": qT_f8,
                "xT": xT_np,
                "xsA": xs_bf,
                "ones": ones_np,
                "xsB": np.ascontiguousarray(np.broadcast_to(xs_bf, (QB, NPAD))),
            }
        )
    return in_maps


def _run_device(in_maps, trace=False, tmpdir=None):
    from concourse.bass_utils import run_bass_kernel_spmd

    nc = _get_compiled()
    kwargs = {}
    if trace:
        kwargs = {"trace": True, "tmpdir": tmpdir}
    return run_bass_kernel_spmd(nc, in_maps, core_ids=list(range(NCORES)), **kwargs)


def _merge_host(maps_u8, q, X, k):
    """maps_u8: [NCORES, B, NPAD] uint8 view of the fp8e5 survivor map.
    Returns [B, 1, D] f32: mean of the exact top-k neighbor rows."""
    xsq_all = np.einsum("nd,nd->n", X, X, dtype=np.float32)
    # survivor = positive fp8e5 byte (0x00 < b < 0x80), real column only
    alive = (maps_u8 > 0) & (maps_u8 < 0x80)
    alive[:, :, NSHARD:] = False
    core_i, q_i, n_i = np.nonzero(alive)
    g_i = core_i * NSHARD + n_i  # global X row
    order = np.argsort(q_i, kind="stable")
    q_s, g_s = q_i[order], g_i[order]
    counts = np.bincount(q_s, minlength=B)
    offs = np.concatenate([[0], np.cumsum(counts)])

    out = np.empty((B, D), dtype=np.float32)
    M = int(counts.max()) if counts.size else 0
    lowq = np.nonzero(counts < k)[0]
    for qi in lowq:  # safety net: brute-force deficient queries (expected: none)
        d2 = xsq_all - 2.0 * (X @ q[qi])
        idx = np.argpartition(d2, k - 1)[:k]
        out[qi] = X[idx].mean(axis=0, dtype=np.float32)

    # vectorized exact re-rank in query chunks
    CH = 256
    pad_idx = np.zeros((B, M), dtype=np.int64)
    valid = np.zeros((B, M), dtype=bool)
    for qi in range(B):
        c = counts[qi]
        if c:
            pad_idx[qi, :c] = g_s[offs[qi] : offs[qi] + c]
            valid[qi, :c] = True
    for i in range(0, B, CH):
        sl = slice(i, i + CH)
        cid = pad_idx[sl]  # [CH, M]
        Xg = X[cid]  # [CH, M, D]
        d2 = xsq_all[cid] - 2.0 * np.einsum(
            "cmd,cd->cm", Xg, q[sl], dtype=np.float32, optimize=True
        )
        d2[~valid[sl]] = np.inf
        for j in range(cid.shape[0]):
            qi = i + j
            if counts[qi] < k:
                continue  # already brute-forced
            win = np.argpartition(d2[j], k - 1)[:k]
            out[qi] = Xg[j, win].mean(axis=0, dtype=np.float32)
    return out.reshape(B, 1, D)


def kernel(x_enc, X_fit, n_neighbors, _trace=False, _tmpdir=None):
    q = np.asarray(x_enc, dtype=np.float32).reshape(B, D)
    X = np.asarray(X_fit, dtype=np.float32)
    k = int(n_neighbors)
    in_maps = _prepare_inputs(q, X)
    res = _run_device(in_maps, trace=_trace, tmpdir=_tmpdir)
    maps = np.stack(
        [res.results[c]["smap"].view(np.uint8) for c in range(NCORES)]
    ).reshape(NCORES, B, NPAD)
    out = _merge_host(maps, q, X, k)
    if _trace:
        return out, res
    return out
